# revision 16
# baseline (speedup 1.0000x reference)
"""Trainium2 Bass kernel for 16-head causal self-attention with RoPE.

Problem: x:[2,2048,2048] -> MHA(wq,wk,wv,wo, causal mask, RoPE) -> [2,2048,2048].

Sharding (8 NeuronCores): core = b*4 + g, where b in {0,1} is the batch
(data parallel) and g in {0..3} is a head group of 4 heads (tensor parallel
over the 16 heads / 2048 channels: group g owns channels [g*512, (g+1)*512)).

All matmul operands are bf16 (PSUM accumulation stays f32); q/k/v stay
SBUF-resident between the projection and attention phases, and all DRAM
operands use partition-major host layouts so each logical load is ONE big
DMA (the HWDGE issues DMAs at only ~1.6/us, so DMA count dominates DMA
cost).  RoPE uses a de-interleaved head-channel layout (host permutes
wq/wk columns, bias and the cos/sin tables): rotate-half becomes a single
half-swap matmul with the sign folded into the sin table.
Each core:
  phase A: stream xT in 512-seq chunks; qT/kT = (x @ wq_loc.T).T + RoPE,
           v = x @ wv_loc.T, all kept in SBUF (bf16).
  phase B: per head, scoresT tiles [sk,sq] = kT-slices @ qT-chunk; pairs of
           k-tiles share one 2-bank PSUM tile so one exp on ScalarE covers
           both (amortizes the ~185ns/instruction ACT overhead); causal:
           strictly-upper tiles skipped, diagonal 128-blocks masked by
           multiply; PV and the denominator (ones-matmul) accumulate in
           PSUM; scores matmuls run 2 pairs ahead of the exp WAR chain.
  phase C: partial out = ao @ wo_loc.T -> DRAM (bf16), one DMA per s-tile
Host: out[b] = sum of the 4 group partials + bo.
"""

import math
import sys

sys.path.insert(0, "/opt/trn_rl_repo")

import numpy as np

N_CORES = 8
B, S, D = 2, 2048, 2048
H, DH = 16, 128
G = 4                 # head groups (tensor-parallel factor per batch)
HPG = H // G          # heads per group = 4
CW = HPG * DH         # channels per group = 512
NT = S // 128         # 16 contraction tiles
SC = 512              # free-dim chunk (one PSUM bank of fp32)
NQ = S // SC          # 4 s-chunks

_NC_CACHE: dict = {}


def build_attn_nc(iters: int = 1, phases: int = 3):
    """Build + compile the Bass module (same program for all 8 cores)."""
    import concourse.tile as tile
    from concourse import bacc, mybir

    f32 = mybir.dt.float32
    bf16 = mybir.dt.bfloat16
    AF = mybir.ActivationFunctionType
    SCALE = 1.0 / math.sqrt(DH)

    nc = bacc.Bacc("TRN2", target_bir_lowering=False, debug=False,
                   num_devices=N_CORES)

    # partition-major layouts: [...][128 partitions][d-block][inner]
    xTt = nc.dram_tensor("xTt", [128, NT, S], bf16, kind="ExternalInput").ap()
    wqt = nc.dram_tensor("wqt", [128, NT, CW], bf16,
                         kind="ExternalInput").ap()
    wkt = nc.dram_tensor("wkt", [128, NT, CW], bf16,
                         kind="ExternalInput").ap()
    wvt = nc.dram_tensor("wvt", [128, NT, CW], bf16,
                         kind="ExternalInput").ap()
    wot = nc.dram_tensor("wot", [128, HPG, D], bf16,
                         kind="ExternalInput").ap()
    bqk = nc.dram_tensor("bqk", [128, 2 * HPG], f32,
                         kind="ExternalInput").ap()
    bvb = nc.dram_tensor("bvb", [128, CW], f32, kind="ExternalInput").ap()
    cosd = nc.dram_tensor("cosd", [DH, S], bf16, kind="ExternalInput").ap()
    sind = nc.dram_tensor("sind", [DH, S], bf16, kind="ExternalInput").ap()
    # [128, 0:128] = half-swap rotation matrix, [128, 128:256] = all-ones
    ptones = nc.dram_tensor("ptones", [128, 256], bf16,
                            kind="ExternalInput").ap()
    mskT = nc.dram_tensor("mskT", [128, 4, SC], bf16,
                          kind="ExternalInput").ap()

    out = nc.dram_tensor("out", [S, D], bf16, kind="ExternalOutput").ap()

    with tile.TileContext(nc) as tc:
        for it in range(iters):
            # persistent across phases: q/k/v (bf16) + attention output
            with tc.tile_pool(name="qkv", bufs=1) as qkv, \
                 tc.tile_pool(name="aop", bufs=1) as aop, \
                 tc.tile_pool(name="cnst", bufs=1) as cnst:
                qT = [qkv.tile([DH, S], bf16, name=f"qT{h}_{it}", tag=f"qT{h}")
                      for h in range(HPG)]
                kT = [qkv.tile([DH, S], bf16, name=f"kT{h}_{it}", tag=f"kT{h}")
                      for h in range(HPG)]
                vT = [qkv.tile([128, CW], bf16, name=f"vT{t}_{it}",
                               tag=f"vT{t}") for t in range(NT)]
                aoT = aop.tile([128, HPG * S], bf16, name=f"aoT_{it}",
                               tag="aoT")
                pto_sb = cnst.tile([128, 256], bf16, name=f"pto_{it}",
                                   tag="pto")
                nc.scalar.dma_start(pto_sb[:], ptones[:])
                pt_sb = pto_sb[:, 0:128]
                ones_sb = pto_sb[:, 128:256]

                # ------------- phase A: projections + RoPE --------------
                with tc.tile_pool(name="wpool", bufs=1) as wpool, \
                     tc.tile_pool(name="xpool", bufs=2) as xpool, \
                     tc.tile_pool(name="x0pool", bufs=1) as x0pool, \
                     tc.tile_pool(name="cspool", bufs=1) as cspool, \
                     tc.tile_pool(name="cnstA", bufs=1) as cnstA, \
                     tc.tile_pool(name="prawp", bufs=4) as prawp, \
                     tc.tile_pool(name="wkp", bufs=4) as wkp, \
                     tc.tile_pool(name="psA", bufs=6, space="PSUM") as psA, \
                     tc.tile_pool(name="psR", bufs=2, space="PSUM") as psR:
                    bqk_sb = cnstA.tile([128, 2 * HPG], f32,
                                        name=f"bqk_{it}", tag="bqk")
                    nc.scalar.dma_start(bqk_sb[:], bqk[:])
                    bvb_sb = cnstA.tile([128, CW], f32, name=f"bvb{it}",
                                        tag="bvb")
                    nc.scalar.dma_start(bvb_sb[:], bvb[:])
                    cos_sb = cspool.tile([DH, S], bf16, name=f"cos_{it}",
                                         tag="cos")
                    sin_sb = cspool.tile([DH, S], bf16, name=f"sin_{it}",
                                         tag="sin")

                    # weights as half tiles (8 d-blocks each) so the first
                    # matmuls only wait on half the weight bytes
                    HB = NT // 2
                    w_sb = {}

                    def wslice(nm, d, c0, c1):
                        nb = NT // len(w_sb[nm])
                        t = w_sb[nm][d // nb]
                        return t[:, (d % nb) * CW + c0:(d % nb) * CW + c1]

                    # chunk-0 x as 4 quarter tiles; later chunks 1 DMA each
                    def load_x_chunk(qi):
                        t = xpool.tile([128, NT * SC], bf16,
                                       name=f"xq{qi}_{it}", tag="xqf")
                        nc.sync.dma_start(
                            t[:], xTt[:, :, qi * SC:(qi + 1) * SC])
                        return [t], NT

                    def xsl(xq, d, a, b):
                        ts, P = xq
                        return ts[d // P][:, (d % P) * SC + a:
                                          (d % P) * SC + b]

                    w_sb["q"] = []
                    QB = NT // 4
                    xq0a = []
                    for qb in range(4):
                        t = wpool.tile([128, QB * CW], bf16,
                                       name=f"wq{qb}_{it}", tag=f"wq{qb}")
                        nc.sync.dma_start(
                            t[:], wqt[:, qb * QB:(qb + 1) * QB, :])
                        w_sb["q"].append(t)
                        tx = x0pool.tile([128, QB * SC], bf16,
                                         name=f"xq0_{qb}_{it}",
                                         tag=f"xq{qb}_4")
                        nc.sync.dma_start(
                            tx[:], xTt[:, qb * QB:(qb + 1) * QB, 0:SC])
                        xq0a.append(tx)
                    xq0 = (xq0a, QB)

                    for nm, dram in (("k", wkt), ("v", wvt)):
                        w_sb[nm] = []
                        for hb in range(2):
                            t = wpool.tile([128, HB * CW], bf16,
                                           name=f"w{nm}{hb}_{it}",
                                           tag=f"w{nm}{hb}")
                            nc.sync.dma_start(
                                t[:], dram[:, hb * HB:(hb + 1) * HB, :])
                            w_sb[nm].append(t)
                        if nm == "k":
                            nc.sync.dma_start(cos_sb[:], cosd[:])
                            nc.sync.dma_start(sin_sb[:], sind[:])

                    for qi in range(NQ):
                        S0 = qi * SC
                        xq = xq0 if qi == 0 else load_x_chunk(qi)
                        for nm, boff, outt in (("q", 0, qT), ("k", HPG, kT)):
                            for cp in range(0, HPG, 2):
                                psa = psA.tile([128, SC], f32,
                                               name=f"ps{nm}{cp}_{qi}_{it}",
                                               tag="ps")
                                psb = psA.tile([128, SC], f32,
                                               name=f"ps{nm}{cp+1}_{qi}_{it}",
                                               tag="ps")
                                for d in range(NT):
                                    nc.tensor.matmul(
                                        psa[:],
                                        wslice(nm, d, cp * DH,
                                               (cp + 1) * DH),
                                        xsl(xq, d, 0, SC),
                                        start=(d == 0), stop=(d == NT - 1))
                                    nc.tensor.matmul(
                                        psb[:],
                                        wslice(nm, d, (cp + 1) * DH,
                                               (cp + 2) * DH),
                                        xsl(xq, d, 0, SC),
                                        start=(d == 0), stop=(d == NT - 1))
                                for ct, ps in ((cp, psa), (cp + 1, psb)):
                                    # drain PSUM + bias on ScalarE
                                    praw = prawp.tile(
                                        [128, SC], bf16,
                                        name=f"praw{nm}{ct}_{qi}_{it}",
                                        tag="praw")
                                    nc.scalar.activation(
                                        praw[:], ps[:], AF.Identity,
                                        bias=bqk_sb[:, boff + ct:
                                                    boff + ct + 1],
                                        scale=1.0)
                                    # rotate-half: half-swap matmul (channel
                                    # layout is de-interleaved; sign folded
                                    # into the sin table)
                                    psr = psR.tile([128, SC], f32,
                                                   name=f"psr{nm}{ct}"
                                                        f"_{qi}_{it}",
                                                   tag="psr")
                                    nc.tensor.matmul(psr[:], pt_sb, praw[:],
                                                     start=True, stop=True)
                                    m1 = wkp.tile([128, SC], bf16,
                                                  name=f"m1{nm}{ct}_{qi}_{it}",
                                                  tag="m1")
                                    nc.vector.tensor_mul(
                                        m1[:], praw[:],
                                        cos_sb[:, S0:S0 + SC])
                                    m2 = wkp.tile([128, SC], bf16,
                                                  name=f"m2{nm}{ct}_{qi}_{it}",
                                                  tag="m2")
                                    nc.vector.tensor_mul(
                                        m2[:], psr[:],
                                        sin_sb[:, S0:S0 + SC])
                                    nc.vector.tensor_add(
                                        outt[ct][:, S0:S0 + SC],
                                        m1[:], m2[:])
                        for sp in range(0, 4, 2):
                            psa = psA.tile([128, SC], f32,
                                           name=f"psv{sp}_{qi}_{it}",
                                           tag="ps")
                            psb = psA.tile([128, SC], f32,
                                           name=f"psv{sp+1}_{qi}_{it}",
                                           tag="ps")
                            for d in range(NT):
                                nc.tensor.matmul(
                                    psa[:],
                                    xsl(xq, d, sp * 128, (sp + 1) * 128),
                                    wslice("v", d, 0, CW),
                                    start=(d == 0), stop=(d == NT - 1))
                                nc.tensor.matmul(
                                    psb[:],
                                    xsl(xq, d, (sp + 1) * 128,
                                        (sp + 2) * 128),
                                    wslice("v", d, 0, CW),
                                    start=(d == 0), stop=(d == NT - 1))
                            for st, ps in ((sp, psa), (sp + 1, psb)):
                                nc.vector.tensor_add(
                                    vT[qi * 4 + st][:], ps[:], bvb_sb[:])

                # ---------------- phase B: attention -------------------
                if phases < 2:
                    continue
                with tc.tile_pool(name="wopool", bufs=1) as wopool, \
                     tc.tile_pool(name="mskpool", bufs=1) as mpool:
                    msk_sb = mpool.tile([128, 4 * SC], bf16,
                                        name=f"msk_{it}", tag="msk")
                    nc.sync.dma_start(msk_sb[:], mskT[:])
                    wo_sb = wopool.tile([128, HPG * D], bf16,
                                        name=f"wo_{it}", tag="wo")
                    nc.sync.dma_start(wo_sb[:], wot[:])

                    with tc.tile_pool(name="atpool", bufs=3) as atpool, \
                         tc.tile_pool(name="recpool", bufs=2) as recpool, \
                         tc.tile_pool(name="psS", bufs=3, space="PSUM") as psS, \
                         tc.tile_pool(name="psO", bufs=1, space="PSUM") as psO:
                        for h in range(HPG):
                            hs = slice(h * DH, (h + 1) * DH)
                            for c in range(NQ):
                                q0 = c * SC
                                ntile = 4 * c + 4
                                npair = ntile // 2
                                qh = qT[h][:, q0:q0 + SC]
                                oT = psO.tile([DH, SC], f32,
                                              name=f"oT{h}{c}_{it}", tag="oT")
                                dn = psO.tile([128, SC], f32,
                                              name=f"dn{h}{c}_{it}", tag="dn")

                                def n0_of(t_):
                                    rr = t_ - 4 * c
                                    return rr * 128 if rr > 0 else 0

                                pend = {}

                                def emit_scores_pair(j):
                                    sps2 = psS.tile(
                                        [128, 2 * SC], f32,
                                        name=f"sps{h}{c}{j}_{it}", tag="sps2")
                                    pend[j] = sps2
                                    for u in (0, 1):
                                        t_ = 2 * j + u
                                        n0 = n0_of(t_)
                                        lo = u * SC
                                        nc.tensor.matmul(
                                            sps2[:, lo + n0:lo + SC],
                                            kT[h][:, t_ * 128:(t_ + 1) * 128],
                                            qh[:, n0:],
                                            start=True, stop=True)

                                for j0 in range(min(2, npair)):
                                    emit_scores_pair(j0)
                                for j in range(npair):
                                    sps2 = pend.pop(j)
                                    at2 = atpool.tile(
                                        [128, 2 * SC], bf16,
                                        name=f"at{h}{c}{j}_{it}", tag="at2")
                                    if 2 * j >= 4 * c:
                                        # diagonal pair: exp exact [n0:]
                                        # ranges (two instructions)
                                        for u in (0, 1):
                                            n0u = n0_of(2 * j + u)
                                            nc.scalar.activation(
                                                at2[:, u * SC + n0u:
                                                    (u + 1) * SC],
                                                sps2[:, u * SC + n0u:
                                                     (u + 1) * SC],
                                                AF.Exp, bias=0.0, scale=SCALE)
                                    else:
                                        nc.scalar.activation(
                                            at2[:], sps2[:],
                                            AF.Exp, bias=0.0, scale=SCALE)
                                    for u in (0, 1):
                                        t_ = 2 * j + u
                                        rr = t_ - 4 * c
                                        n0 = n0_of(t_)
                                        if rr >= 0:
                                            nc.vector.tensor_mul(
                                                at2[:, u * SC + n0:
                                                    u * SC + n0 + 128],
                                                at2[:, u * SC + n0:
                                                    u * SC + n0 + 128],
                                                msk_sb[:, rr * SC + n0:
                                                       rr * SC + n0 + 128])
                                    if j + 2 < npair:
                                        emit_scores_pair(j + 2)
                                    for u in (0, 1):
                                        t_ = 2 * j + u
                                        n0 = n0_of(t_)
                                        nc.tensor.matmul(
                                            oT[:, n0:],
                                            vT[t_][:, hs],
                                            at2[:, u * SC + n0:(u + 1) * SC],
                                            start=(t_ == 0),
                                            stop=(t_ == ntile - 1),
                                            skip_group_check=True)
                                        nc.tensor.matmul(
                                            dn[:, n0:], ones_sb,
                                            at2[:, u * SC + n0:(u + 1) * SC],
                                            start=(t_ == 0),
                                            stop=(t_ == ntile - 1),
                                            skip_group_check=True)
                                rec = recpool.tile([128, SC], f32,
                                                   name=f"rec{h}{c}_{it}",
                                                   tag="rec")
                                nc.vector.reciprocal(rec[:], dn[:])
                                nc.vector.tensor_mul(
                                    aoT[:, h * S + q0:h * S + q0 + SC],
                                    oT[:], rec[:])

                    # ------------ phase C: output projection ------------
                    if phases < 3:
                        for st in range(4):
                            nc.sync.dma_start(
                                out[st * 128:(st + 1) * 128, :],
                                aoT[:, st * D:(st + 1) * D])
                        continue
                    with tc.tile_pool(name="outpool", bufs=3) as outpool, \
                         tc.tile_pool(name="psC", bufs=4, space="PSUM") as psC:
                        for st in range(NT):
                            ops = []
                            for dp in range(2):
                                op = psC.tile([128, 2 * SC], f32,
                                              name=f"op{st}{dp}_{it}",
                                              tag="op")
                                ops.append(op)
                            for hh in range(HPG):
                                lhs = aoT[:, hh * S + st * 128:
                                          hh * S + (st + 1) * 128]
                                for dc in range(4):
                                    nc.tensor.matmul(
                                        ops[dc // 2][:, (dc % 2) * SC:
                                                     (dc % 2 + 1) * SC],
                                        lhs,
                                        wo_sb[:, hh * D + dc * SC:
                                              hh * D + (dc + 1) * SC],
                                        start=(hh == 0), stop=(hh == HPG - 1),
                                        skip_group_check=True)
                            ot = outpool.tile([128, D], bf16,
                                              name=f"ot{st}_{it}", tag="ot")
                            for dp in range(2):
                                nc.scalar.activation(
                                    ot[:, dp * 2 * SC:(dp + 1) * 2 * SC],
                                    ops[dp][:], AF.Copy, bias=0.0, scale=1.0)
                            nc.sync.dma_start(
                                out[st * 128:(st + 1) * 128, :], ot[:])
    nc.compile()
    return nc


def _deinter_perm():
    """Per-head de-interleave: new j<64 -> old 2j (even), j>=64 -> old
    2(j-64)+1 (odd)."""
    p = np.empty(DH, np.int64)
    p[:64] = np.arange(64) * 2
    p[64:] = np.arange(64) * 2 + 1
    return p


def host_prep(inputs: dict) -> list:
    """Build per-core input maps (host-side sharding + relayout + bf16)."""
    import ml_dtypes

    bf16 = ml_dtypes.bfloat16
    x = np.asarray(inputs["x"], dtype=np.float32)
    wq = np.asarray(inputs["wq"], dtype=np.float32)
    wk = np.asarray(inputs["wk"], dtype=np.float32)
    wv = np.asarray(inputs["wv"], dtype=np.float32)
    wo = np.asarray(inputs["wo"], dtype=np.float32)
    bq = np.asarray(inputs["bq"], dtype=np.float32)
    bk = np.asarray(inputs["bk"], dtype=np.float32)
    bv = np.asarray(inputs["bv"], dtype=np.float32)
    mask = np.asarray(inputs["mask"])

    perm = _deinter_perm()
    inv = 1.0 / (10000.0 ** (np.arange(0, DH, 2, dtype=np.float64) / DH))
    ang = np.arange(S, dtype=np.float64)[:, None] * inv[None, :]  # [S, 64]
    # de-interleaved tables [DH, S]: rows j<64 and j>=64 share angle j%64;
    # sin sign-folded: row j<64 gets -sin (pairs with psr[j] = praw[64+j])
    cosd = np.empty((DH, S), np.float32)
    sind = np.empty((DH, S), np.float32)
    cosd[:64] = np.cos(ang).T
    cosd[64:] = cosd[:64]
    sind[:64] = -np.sin(ang).T
    sind[64:] = -sind[:64]

    # half-swap rotation matrix (de-interleaved rotate-half, sign in sind)
    # psr = PT.T @ praw with PT[j, 64+j] = PT[64+j, j] = 1
    PT = np.zeros((128, 128), np.float32)
    PT[np.arange(64), np.arange(64) + 64] = 1.0
    PT[np.arange(64) + 64, np.arange(64)] = 1.0
    ptones = np.concatenate([PT, np.ones((128, 128), np.float32)], axis=1)

    m2 = mask[0, 0]
    # keep-mask diag blocks, partition-major [128, 4, SC]
    mskT = np.zeros((128, 4, SC), np.float32)
    for rr in range(4):
        # keep[i, j] = not masked(q=j, k=rr*128+i)
        mskT[:, rr, :] = (~m2[:SC, rr * 128:(rr + 1) * 128]).T
    # per-head column de-interleave over the full CW channel range
    permCW = np.concatenate([ct * DH + perm for ct in range(HPG)])

    def pmaj(a, nt):
        # [nt*128, F] -> [128, nt, F]
        F = a.shape[1]
        return np.ascontiguousarray(
            a.reshape(nt, 128, F).transpose(1, 0, 2)).astype(bf16)

    xTb = [pmaj(np.ascontiguousarray(x[b].T), NT) for b in range(B)]
    in_maps = []
    for core in range(N_CORES):
        b, g = divmod(core, G)
        c0 = g * CW
        wq_g = wq[c0:c0 + CW, :][permCW, :]   # rows = out channels
        wk_g = wk[c0:c0 + CW, :][permCW, :]
        bq_g = bq[c0:c0 + CW][permCW].reshape(HPG, DH)
        bk_g = bk[c0:c0 + CW][permCW].reshape(HPG, DH)
        in_maps.append({
            "xTt": xTb[b],
            "wqt": pmaj(np.ascontiguousarray(wq_g.T), NT),
            "wkt": pmaj(np.ascontiguousarray(wk_g.T), NT),
            "wvt": pmaj(np.ascontiguousarray(wv[c0:c0 + CW, :].T), NT),
            "wot": pmaj(np.ascontiguousarray(wo[:, c0:c0 + CW].T), HPG),
            "bqk": np.ascontiguousarray(
                np.concatenate([bq_g, bk_g], axis=0).T),
            "bvb": np.ascontiguousarray(
                np.broadcast_to(bv[c0:c0 + CW], (128, CW))),
            "cosd": cosd.astype(bf16),
            "sind": sind.astype(bf16),
            "ptones": ptones.astype(bf16),
            "mskT": mskT.astype(bf16),
        })
    return in_maps


def _get_nc():
    if "nc" not in _NC_CACHE:
        _NC_CACHE["nc"] = build_attn_nc(iters=1)
    return _NC_CACHE["nc"]


def kernel(**inputs) -> np.ndarray:
    from concourse.bass_utils import run_bass_kernel_spmd

    nc = _get_nc()
    in_maps = host_prep(inputs)
    res = run_bass_kernel_spmd(nc, in_maps, core_ids=list(range(N_CORES)))
    bo = np.asarray(inputs["bo"], dtype=np.float32)
    outp = np.zeros((B, S, D), np.float32)
    for core in range(N_CORES):
        outp[core // G] += np.asarray(res.results[core]["out"],
                                      dtype=np.float32)
    outp += bo[None, None, :]
    return outp


# revision 21
# speedup vs baseline: 1.3309x; 1.3309x over previous
"""Trainium2 Bass kernel for 16-head causal self-attention with RoPE.

Problem: x:[2,2048,2048] -> MHA(wq,wk,wv,wo, causal mask, RoPE) -> [2,2048,2048].

Sharding (8 NeuronCores): core = b*4 + g, where b in {0,1} is the batch
(data parallel) and g in {0..3} is a head group of 4 heads (tensor parallel
over the 16 heads / 2048 channels: group g owns channels [g*512, (g+1)*512)).

All matmul operands are bf16 (PSUM accumulation stays f32); q/k/v stay
SBUF-resident between the projection and attention phases, and all DRAM
operands use partition-major host layouts so each logical load is ONE big
DMA (the HWDGE issues DMAs at only ~1.6/us, so DMA count dominates DMA
cost).  RoPE uses a de-interleaved head-channel layout (host permutes
wq/wk columns, bias and the cos/sin tables): rotate-half becomes a single
half-swap matmul with the sign folded into the sin table.
Each core:
  phase A: stream xT in 512-seq chunks; qT/kT = (x @ wq_loc.T).T + RoPE,
           v = x @ wv_loc.T, all kept in SBUF (bf16).
  phase B: per head, scoresT tiles [sk,sq] = kT-slices @ qT-chunk; pairs of
           k-tiles share one 2-bank PSUM tile so one exp on ScalarE covers
           both (amortizes the ~185ns/instruction ACT overhead); causal:
           strictly-upper tiles skipped, diagonal 128-blocks masked by
           multiply; PV and the denominator (ones-matmul) accumulate in
           PSUM; scores matmuls run 2 pairs ahead of the exp WAR chain.
  phase C: partial out = ao @ wo_loc.T -> DRAM (bf16), one DMA per s-tile
Host: out[b] = sum of the 4 group partials + bo.
"""

import math
import sys

sys.path.insert(0, "/opt/trn_rl_repo")

import numpy as np

N_CORES = 8
B, S, D = 2, 2048, 2048
H, DH = 16, 128
G = 4                 # head groups (tensor-parallel factor per batch)
HPG = H // G          # heads per group = 4
CW = HPG * DH         # channels per group = 512
NT = S // 128         # 16 contraction tiles
SC = 512              # free-dim chunk (one PSUM bank of fp32)
NQ = S // SC          # 4 s-chunks

_NC_CACHE: dict = {}


def build_attn_nc(iters: int = 1, phases: int = 3):
    """Build + compile the Bass module (same program for all 8 cores)."""
    import concourse.tile as tile
    from concourse import bacc, mybir

    f32 = mybir.dt.float32
    bf16 = mybir.dt.bfloat16
    AF = mybir.ActivationFunctionType
    SCALE = 1.0 / math.sqrt(DH)

    nc = bacc.Bacc("TRN2", target_bir_lowering=False, debug=False,
                   num_devices=N_CORES)

    # partition-major layouts: [...][128 partitions][d-block][inner]
    xTc = nc.dram_tensor("xTc", [NQ, 128, NT * SC], bf16,
                         kind="ExternalInput").ap()
    wqt = nc.dram_tensor("wqt", [128, NT * CW], bf16,
                         kind="ExternalInput").ap()
    wkt = nc.dram_tensor("wkt", [128, NT * CW], bf16,
                         kind="ExternalInput").ap()
    wvt = nc.dram_tensor("wvt", [128, NT * CW], bf16,
                         kind="ExternalInput").ap()
    wot = nc.dram_tensor("wot", [128, HPG * D], bf16,
                         kind="ExternalInput").ap()
    bqk = nc.dram_tensor("bqk", [128, 2 * HPG], f32,
                         kind="ExternalInput").ap()
    bvb = nc.dram_tensor("bvb", [128, CW], f32, kind="ExternalInput").ap()
    cosd = nc.dram_tensor("cosd", [DH, S], bf16, kind="ExternalInput").ap()
    sind = nc.dram_tensor("sind", [DH, S], bf16, kind="ExternalInput").ap()
    # [128, 0:128] = half-swap rotation matrix, [128, 128:256] = all-ones
    ptones = nc.dram_tensor("ptones", [128, 256], bf16,
                            kind="ExternalInput").ap()
    mskT = nc.dram_tensor("mskT", [128, 4, SC], bf16,
                          kind="ExternalInput").ap()

    out = nc.dram_tensor("out", [S, D], bf16, kind="ExternalOutput").ap()

    with tile.TileContext(nc) as tc:
        for it in range(iters):
            # persistent across phases: q/k/v (bf16) + attention output
            with tc.tile_pool(name="qkv", bufs=1) as qkv, \
                 tc.tile_pool(name="aop", bufs=1) as aop, \
                 tc.tile_pool(name="cnst", bufs=1) as cnst:
                qT = [qkv.tile([DH, S], bf16, name=f"qT{h}_{it}", tag=f"qT{h}")
                      for h in range(HPG)]
                kT = [qkv.tile([DH, S], bf16, name=f"kT{h}_{it}", tag=f"kT{h}")
                      for h in range(HPG)]
                vT = [qkv.tile([128, CW], bf16, name=f"vT{t}_{it}",
                               tag=f"vT{t}") for t in range(NT)]
                aoT = aop.tile([128, HPG * S], bf16, name=f"aoT_{it}",
                               tag="aoT")
                pto_sb = cnst.tile([128, 256], bf16, name=f"pto_{it}",
                                   tag="pto")
                pt_sb = pto_sb[:, 0:128]
                ones_sb = pto_sb[:, 128:256]

                # ------------- phase A: projections + RoPE --------------
                with tc.tile_pool(name="wpool", bufs=1) as wpool, \
                     tc.tile_pool(name="xpool", bufs=2) as xpool, \
                     tc.tile_pool(name="x0pool", bufs=1) as x0pool, \
                     tc.tile_pool(name="cspool", bufs=1) as cspool, \
                     tc.tile_pool(name="cnstA", bufs=1) as cnstA, \
                     tc.tile_pool(name="prawp", bufs=4) as prawp, \
                     tc.tile_pool(name="wkp", bufs=4) as wkp, \
                     tc.tile_pool(name="psA", bufs=6, space="PSUM") as psA, \
                     tc.tile_pool(name="psR", bufs=2, space="PSUM") as psR:
                    bqk_sb = cnstA.tile([128, 2 * HPG], f32,
                                        name=f"bqk_{it}", tag="bqk")
                    bvb_sb = cnstA.tile([128, CW], f32, name=f"bvb{it}",
                                        tag="bvb")
                    cos_sb = cspool.tile([DH, S], bf16, name=f"cos_{it}",
                                         tag="cos")
                    sin_sb = cspool.tile([DH, S], bf16, name=f"sin_{it}",
                                         tag="sin")

                    # weights as half tiles (8 d-blocks each) so the first
                    # matmuls only wait on half the weight bytes
                    HB = NT // 2
                    w_sb = {}

                    def wslice(nm, d, c0, c1):
                        nb = NT // len(w_sb[nm])
                        t = w_sb[nm][d // nb]
                        return t[:, (d % nb) * CW + c0:(d % nb) * CW + c1]

                    # chunk-0 x as 4 quarter tiles; later chunks 1 DMA each
                    def load_x_chunk(qi):
                        t = xpool.tile([128, NT * SC], bf16,
                                       name=f"xq{qi}_{it}", tag="xqf")
                        nc.sync.dma_start(t[:], xTc[qi])
                        return [t], NT

                    def xsl(xq, d, a, b):
                        ts, P = xq
                        return ts[d // P][:, (d % P) * SC + a:
                                          (d % P) * SC + b]

                    w_sb["q"] = []
                    QB = NT // 4
                    xq0a = []
                    for qb in range(4):
                        t = wpool.tile([128, QB * CW], bf16,
                                       name=f"wq{qb}_{it}", tag=f"wq{qb}")
                        nc.scalar.dma_start(
                            t[:], wqt[:, qb * QB * CW:(qb + 1) * QB * CW])
                        w_sb["q"].append(t)
                        tx = x0pool.tile([128, QB * SC], bf16,
                                         name=f"xq0_{qb}_{it}",
                                         tag=f"xq{qb}_4")
                        nc.sync.dma_start(
                            tx[:], xTc[0][:, qb * QB * SC:
                                          (qb + 1) * QB * SC])
                        xq0a.append(tx)
                    xq0 = (xq0a, QB)
                    nc.scalar.dma_start(bqk_sb[:], bqk[:])
                    nc.scalar.dma_start(pto_sb[:], ptones[:])
                    nc.scalar.dma_start(bvb_sb[:], bvb[:])

                    for nm, dram in (("k", wkt), ("v", wvt)):
                        w_sb[nm] = []
                        for hb in range(2):
                            t = wpool.tile([128, HB * CW], bf16,
                                           name=f"w{nm}{hb}_{it}",
                                           tag=f"w{nm}{hb}")
                            nc.scalar.dma_start(
                                t[:], dram[:, hb * HB * CW:
                                           (hb + 1) * HB * CW])
                            w_sb[nm].append(t)
                        if nm == "k":
                            nc.scalar.dma_start(cos_sb[:], cosd[:])
                            nc.scalar.dma_start(sin_sb[:], sind[:])

                    for qi in range(NQ):
                        S0 = qi * SC
                        xq = xq0 if qi == 0 else load_x_chunk(qi)
                        for nm, boff, outt in (("q", 0, qT), ("k", HPG, kT)):
                            for cp in range(0, HPG, 2):
                                psa = psA.tile([128, SC], f32,
                                               name=f"ps{nm}{cp}_{qi}_{it}",
                                               tag="ps")
                                psb = psA.tile([128, SC], f32,
                                               name=f"ps{nm}{cp+1}_{qi}_{it}",
                                               tag="ps")
                                for d in range(NT):
                                    nc.tensor.matmul(
                                        psa[:],
                                        wslice(nm, d, cp * DH,
                                               (cp + 1) * DH),
                                        xsl(xq, d, 0, SC),
                                        start=(d == 0), stop=(d == NT - 1))
                                    nc.tensor.matmul(
                                        psb[:],
                                        wslice(nm, d, (cp + 1) * DH,
                                               (cp + 2) * DH),
                                        xsl(xq, d, 0, SC),
                                        start=(d == 0), stop=(d == NT - 1))
                                for ct, ps in ((cp, psa), (cp + 1, psb)):
                                    # drain PSUM + bias on ScalarE
                                    praw = prawp.tile(
                                        [128, SC], bf16,
                                        name=f"praw{nm}{ct}_{qi}_{it}",
                                        tag="praw")
                                    nc.scalar.activation(
                                        praw[:], ps[:], AF.Identity,
                                        bias=bqk_sb[:, boff + ct:
                                                    boff + ct + 1],
                                        scale=1.0)
                                    # rotate-half: half-swap matmul (channel
                                    # layout is de-interleaved; sign folded
                                    # into the sin table)
                                    psr = psR.tile([128, SC], f32,
                                                   name=f"psr{nm}{ct}"
                                                        f"_{qi}_{it}",
                                                   tag="psr")
                                    nc.tensor.matmul(psr[:], pt_sb, praw[:],
                                                     start=True, stop=True)
                                    m1 = wkp.tile([128, SC], bf16,
                                                  name=f"m1{nm}{ct}_{qi}_{it}",
                                                  tag="m1")
                                    nc.vector.tensor_mul(
                                        m1[:], praw[:],
                                        cos_sb[:, S0:S0 + SC])
                                    m2 = wkp.tile([128, SC], bf16,
                                                  name=f"m2{nm}{ct}_{qi}_{it}",
                                                  tag="m2")
                                    nc.vector.tensor_mul(
                                        m2[:], psr[:],
                                        sin_sb[:, S0:S0 + SC])
                                    nc.vector.tensor_add(
                                        outt[ct][:, S0:S0 + SC],
                                        m1[:], m2[:])
                        for sp in range(0, 4, 2):
                            psa = psA.tile([128, SC], f32,
                                           name=f"psv{sp}_{qi}_{it}",
                                           tag="ps")
                            psb = psA.tile([128, SC], f32,
                                           name=f"psv{sp+1}_{qi}_{it}",
                                           tag="ps")
                            for d in range(NT):
                                nc.tensor.matmul(
                                    psa[:],
                                    xsl(xq, d, sp * 128, (sp + 1) * 128),
                                    wslice("v", d, 0, CW),
                                    start=(d == 0), stop=(d == NT - 1))
                                nc.tensor.matmul(
                                    psb[:],
                                    xsl(xq, d, (sp + 1) * 128,
                                        (sp + 2) * 128),
                                    wslice("v", d, 0, CW),
                                    start=(d == 0), stop=(d == NT - 1))
                            for st, ps in ((sp, psa), (sp + 1, psb)):
                                nc.vector.tensor_add(
                                    vT[qi * 4 + st][:], ps[:], bvb_sb[:])

                # ---------------- phase B: attention -------------------
                if phases < 2:
                    continue
                with tc.tile_pool(name="wopool", bufs=1) as wopool, \
                     tc.tile_pool(name="mskpool", bufs=1) as mpool:
                    msk_sb = mpool.tile([128, 4 * SC], bf16,
                                        name=f"msk_{it}", tag="msk")
                    nc.scalar.dma_start(msk_sb[:], mskT[:])
                    wo_sb = wopool.tile([128, HPG * D], bf16,
                                        name=f"wo_{it}", tag="wo")
                    nc.scalar.dma_start(wo_sb[:], wot[:])

                    with tc.tile_pool(name="atpool", bufs=3) as atpool, \
                         tc.tile_pool(name="recpool", bufs=2) as recpool, \
                         tc.tile_pool(name="psS", bufs=3, space="PSUM") as psS, \
                         tc.tile_pool(name="psO", bufs=1, space="PSUM") as psO:
                        for h in range(HPG):
                            hs = slice(h * DH, (h + 1) * DH)
                            for c in range(NQ):
                                q0 = c * SC
                                ntile = 4 * c + 4
                                npair = ntile // 2
                                qh = qT[h][:, q0:q0 + SC]
                                oT = psO.tile([DH, SC], f32,
                                              name=f"oT{h}{c}_{it}", tag="oT")
                                dn = psO.tile([128, SC], f32,
                                              name=f"dn{h}{c}_{it}", tag="dn")

                                def n0_of(t_):
                                    rr = t_ - 4 * c
                                    return rr * 128 if rr > 0 else 0

                                pend = {}

                                def emit_scores_pair(j):
                                    sps2 = psS.tile(
                                        [128, 2 * SC], f32,
                                        name=f"sps{h}{c}{j}_{it}", tag="sps2")
                                    pend[j] = sps2
                                    for u in (0, 1):
                                        t_ = 2 * j + u
                                        n0 = n0_of(t_)
                                        lo = u * SC
                                        nc.tensor.matmul(
                                            sps2[:, lo + n0:lo + SC],
                                            kT[h][:, t_ * 128:(t_ + 1) * 128],
                                            qh[:, n0:],
                                            start=True, stop=True)

                                for j0 in range(min(2, npair)):
                                    emit_scores_pair(j0)
                                for j in range(npair):
                                    sps2 = pend.pop(j)
                                    at2 = atpool.tile(
                                        [128, 2 * SC], bf16,
                                        name=f"at{h}{c}{j}_{it}", tag="at2")
                                    if 2 * j >= 4 * c:
                                        # diagonal pair: exp exact [n0:]
                                        # ranges (two instructions)
                                        for u in (0, 1):
                                            n0u = n0_of(2 * j + u)
                                            nc.scalar.activation(
                                                at2[:, u * SC + n0u:
                                                    (u + 1) * SC],
                                                sps2[:, u * SC + n0u:
                                                     (u + 1) * SC],
                                                AF.Exp, bias=0.0, scale=SCALE)
                                    else:
                                        nc.scalar.activation(
                                            at2[:], sps2[:],
                                            AF.Exp, bias=0.0, scale=SCALE)
                                    for u in (0, 1):
                                        t_ = 2 * j + u
                                        rr = t_ - 4 * c
                                        n0 = n0_of(t_)
                                        if rr >= 0:
                                            nc.vector.tensor_mul(
                                                at2[:, u * SC + n0:
                                                    u * SC + n0 + 128],
                                                at2[:, u * SC + n0:
                                                    u * SC + n0 + 128],
                                                msk_sb[:, rr * SC + n0:
                                                       rr * SC + n0 + 128])
                                    if j + 2 < npair:
                                        emit_scores_pair(j + 2)
                                    for u in (0, 1):
                                        t_ = 2 * j + u
                                        n0 = n0_of(t_)
                                        nc.tensor.matmul(
                                            oT[:, n0:],
                                            vT[t_][:, hs],
                                            at2[:, u * SC + n0:(u + 1) * SC],
                                            start=(t_ == 0),
                                            stop=(t_ == ntile - 1),
                                            skip_group_check=True)
                                        nc.tensor.matmul(
                                            dn[:, n0:], ones_sb,
                                            at2[:, u * SC + n0:(u + 1) * SC],
                                            start=(t_ == 0),
                                            stop=(t_ == ntile - 1),
                                            skip_group_check=True)
                                rec = recpool.tile([128, SC], f32,
                                                   name=f"rec{h}{c}_{it}",
                                                   tag="rec")
                                nc.vector.reciprocal(rec[:], dn[:])
                                nc.vector.tensor_mul(
                                    aoT[:, h * S + q0:h * S + q0 + SC],
                                    oT[:], rec[:])

                    # ------------ phase C: output projection ------------
                    if phases < 3:
                        for st in range(4):
                            nc.sync.dma_start(
                                out[st * 128:(st + 1) * 128, :],
                                aoT[:, st * D:(st + 1) * D])
                        continue
                    with tc.tile_pool(name="outpool", bufs=3) as outpool, \
                         tc.tile_pool(name="psC", bufs=4, space="PSUM") as psC:
                        for st in range(NT):
                            ops = []
                            for dp in range(2):
                                op = psC.tile([128, 2 * SC], f32,
                                              name=f"op{st}{dp}_{it}",
                                              tag="op")
                                ops.append(op)
                            for hh in range(HPG):
                                lhs = aoT[:, hh * S + st * 128:
                                          hh * S + (st + 1) * 128]
                                for dc in range(4):
                                    nc.tensor.matmul(
                                        ops[dc // 2][:, (dc % 2) * SC:
                                                     (dc % 2 + 1) * SC],
                                        lhs,
                                        wo_sb[:, hh * D + dc * SC:
                                              hh * D + (dc + 1) * SC],
                                        start=(hh == 0), stop=(hh == HPG - 1),
                                        skip_group_check=True)
                            ot = outpool.tile([128, D], bf16,
                                              name=f"ot{st}_{it}", tag="ot")
                            for dp in range(2):
                                nc.scalar.activation(
                                    ot[:, dp * 2 * SC:(dp + 1) * 2 * SC],
                                    ops[dp][:], AF.Copy, bias=0.0, scale=1.0)
                            # store on the scalar queue: keeps the sync queue
                            # free so the next iteration's weight/x loads can
                            # issue during phases B/C
                            nc.scalar.dma_start(
                                out[st * 128:(st + 1) * 128, :], ot[:])
    nc.compile()
    return nc


def _deinter_perm():
    """Per-head de-interleave: new j<64 -> old 2j (even), j>=64 -> old
    2(j-64)+1 (odd)."""
    p = np.empty(DH, np.int64)
    p[:64] = np.arange(64) * 2
    p[64:] = np.arange(64) * 2 + 1
    return p


def host_prep(inputs: dict) -> list:
    """Build per-core input maps (host-side sharding + relayout + bf16)."""
    import ml_dtypes

    bf16 = ml_dtypes.bfloat16
    x = np.asarray(inputs["x"], dtype=np.float32)
    wq = np.asarray(inputs["wq"], dtype=np.float32)
    wk = np.asarray(inputs["wk"], dtype=np.float32)
    wv = np.asarray(inputs["wv"], dtype=np.float32)
    wo = np.asarray(inputs["wo"], dtype=np.float32)
    bq = np.asarray(inputs["bq"], dtype=np.float32)
    bk = np.asarray(inputs["bk"], dtype=np.float32)
    bv = np.asarray(inputs["bv"], dtype=np.float32)
    mask = np.asarray(inputs["mask"])

    perm = _deinter_perm()
    inv = 1.0 / (10000.0 ** (np.arange(0, DH, 2, dtype=np.float64) / DH))
    ang = np.arange(S, dtype=np.float64)[:, None] * inv[None, :]  # [S, 64]
    # de-interleaved tables [DH, S]: rows j<64 and j>=64 share angle j%64;
    # sin sign-folded: row j<64 gets -sin (pairs with psr[j] = praw[64+j])
    cosd = np.empty((DH, S), np.float32)
    sind = np.empty((DH, S), np.float32)
    cosd[:64] = np.cos(ang).T
    cosd[64:] = cosd[:64]
    sind[:64] = -np.sin(ang).T
    sind[64:] = -sind[:64]

    # half-swap rotation matrix (de-interleaved rotate-half, sign in sind)
    # psr = PT.T @ praw with PT[j, 64+j] = PT[64+j, j] = 1
    PT = np.zeros((128, 128), np.float32)
    PT[np.arange(64), np.arange(64) + 64] = 1.0
    PT[np.arange(64) + 64, np.arange(64)] = 1.0
    ptones = np.concatenate([PT, np.ones((128, 128), np.float32)], axis=1)

    m2 = mask[0, 0]
    # keep-mask diag blocks, partition-major [128, 4, SC]
    mskT = np.zeros((128, 4, SC), np.float32)
    for rr in range(4):
        # keep[i, j] = not masked(q=j, k=rr*128+i)
        mskT[:, rr, :] = (~m2[:SC, rr * 128:(rr + 1) * 128]).T
    # per-head column de-interleave over the full CW channel range
    permCW = np.concatenate([ct * DH + perm for ct in range(HPG)])

    def pmaj(a, nt):
        # [nt*128, F] -> [128, nt, F]
        F = a.shape[1]
        return np.ascontiguousarray(
            a.reshape(nt, 128, F).transpose(1, 0, 2)).astype(bf16)

    # chunk-contiguous x: [NQ chunks, 128 partitions, NT*SC] so every
    # chunk load is 128 contiguous 16KB descriptors
    def xlayout(xb):
        xt = np.ascontiguousarray(xb.T).astype(bf16)      # [D, S]
        a = xt.reshape(NT, 128, NQ, SC)                   # [d, p, chunk, s]
        return np.ascontiguousarray(a.transpose(2, 1, 0, 3)
                                    ).reshape(NQ, 128, NT * SC)

    xTb = [xlayout(x[b]) for b in range(B)]
    in_maps = []
    for core in range(N_CORES):
        b, g = divmod(core, G)
        c0 = g * CW
        wq_g = wq[c0:c0 + CW, :][permCW, :]   # rows = out channels
        wk_g = wk[c0:c0 + CW, :][permCW, :]
        bq_g = bq[c0:c0 + CW][permCW].reshape(HPG, DH)
        bk_g = bk[c0:c0 + CW][permCW].reshape(HPG, DH)
        in_maps.append({
            "xTc": xTb[b],
            "wqt": pmaj(np.ascontiguousarray(wq_g.T), NT).reshape(128, -1),
            "wkt": pmaj(np.ascontiguousarray(wk_g.T), NT).reshape(128, -1),
            "wvt": pmaj(np.ascontiguousarray(wv[c0:c0 + CW, :].T),
                        NT).reshape(128, -1),
            "wot": pmaj(np.ascontiguousarray(wo[:, c0:c0 + CW].T),
                        HPG).reshape(128, -1),
            "bqk": np.ascontiguousarray(
                np.concatenate([bq_g, bk_g], axis=0).T),
            "bvb": np.ascontiguousarray(
                np.broadcast_to(bv[c0:c0 + CW], (128, CW))),
            "cosd": cosd.astype(bf16),
            "sind": sind.astype(bf16),
            "ptones": ptones.astype(bf16),
            "mskT": mskT.astype(bf16),
        })
    return in_maps


def _get_nc():
    if "nc" not in _NC_CACHE:
        _NC_CACHE["nc"] = build_attn_nc(iters=1)
    return _NC_CACHE["nc"]


def kernel(**inputs) -> np.ndarray:
    from concourse.bass_utils import run_bass_kernel_spmd

    nc = _get_nc()
    in_maps = host_prep(inputs)
    res = run_bass_kernel_spmd(nc, in_maps, core_ids=list(range(N_CORES)))
    bo = np.asarray(inputs["bo"], dtype=np.float32)
    outp = np.zeros((B, S, D), np.float32)
    for core in range(N_CORES):
        outp[core // G] += np.asarray(res.results[core]["out"],
                                      dtype=np.float32)
    outp += bo[None, None, :]
    return outp
